# revision 4
# baseline (speedup 1.0000x reference)
"""DeformableFeatureAlignment fused Trainium2 kernel, v2.

v2 vs baseline: the DCNv2 bilinear gather uses a d=4 "quad" table
ET4[q] = [E(q), E(q+1), E(q+64), E(q+65)] (x-pair AND row-pair packed per
entry, mod-4096 wrap = rotation-correct), so ONE ap_gather per
(phase, channel-half, tap) fetches all 4 bilinear neighbors: 36 gathers
instead of 72.  Row-collapse edge (floor(y) outside [0,62]) handled by
merging WY1 into the row-0 weight.  fcal/AL/ym in bf16 to fit SBUF.

Sharding: data-parallel over (batch, row-half): core c handles batch c//2,
output rows 32*(c%2) .. +32.  All weights replicated.
"""
import sys

if "/opt/trn_rl_repo" not in sys.path:
    sys.path.insert(0, "/opt/trn_rl_repo")

import numpy as np
from contextlib import ExitStack

import concourse.bass as bass
import concourse.tile as tile
from concourse import bacc, mybir
from concourse.bass_utils import run_bass_kernel_spmd

FP32 = mybir.dt.float32
BF16 = mybir.dt.bfloat16
I32 = mybir.dt.int32
I16 = mybir.dt.int16
AF = mybir.ActivationFunctionType
OP = mybir.AluOpType

B, H, W, C, F = 4, 64, 64, 256, 256
DG, K, KK = 8, 3, 9
NCORES = 8
POS = 32 * W
HALO = 34 * W
NPH = 2
PPOS = POS // NPH
OMF = DG * 3 * KK


# ---------------------------------------------------------------- host prep
def _host_consts():
    om_perm = np.zeros(OMF, np.int64)
    for g in range(DG):
        for k in range(KK):
            om_perm[g * KK + k] = g * 2 * KK + 2 * k
            om_perm[72 + g * KK + k] = g * 2 * KK + 2 * k + 1
            om_perm[144 + g * KK + k] = 144 + g * KK + k
    ky = np.repeat(np.arange(K) - 1, K).astype(np.float32)
    kx = np.tile(np.arange(K) - 1, K).astype(np.float32)
    oh = np.zeros((2, KK, 72, 128), np.float32)
    for t in range(2):
        for k in range(KK):
            for p in range(128):
                g = 4 * t + p // 32
                oh[t, k, g * KK + k, p] = 1.0
    return om_perm, ky, kx, oh


def _prep_weights(attend_w, select_w, offset_w, om_w, om_b, dcn_w, dcn_b):
    om_perm, ky, kx, oh = _host_consts()
    w_att = (attend_w / (H * W)).astype(np.float32)
    wp = np.einsum("co,dof->dcf", offset_w, om_w.reshape(KK, 2 * F, OMF))
    wp = wp.copy()
    wp[:, F:, :] *= 2.0
    wp = wp[:, :, om_perm].astype(np.float32)
    dcn_w9 = dcn_w.reshape(KK, C, F)
    import ml_dtypes
    dcn_wb = dcn_w9.astype(ml_dtypes.bfloat16)
    ohb = oh.astype(ml_dtypes.bfloat16)
    wpb = wp.astype(ml_dtypes.bfloat16)
    return dict(w_att=w_att, w_sel=select_w.astype(np.float32),
                wpb=wpb, dcn_wb=dcn_wb,
                dcn_b=dcn_b.astype(np.float32), ohb=ohb, ky=ky, kx=kx)


def _core_inputs(core, fine, coarse, wd):
    b, half = core // 2, core % 2
    r0 = 32 * half
    fb = np.asarray(fine[b], np.float32).reshape(H * W, C)
    halo = np.zeros((34, W, C), np.float32)
    lo, hi = r0 - 1, r0 + 33
    src_lo, src_hi = max(lo, 0), min(hi, H)
    halo[src_lo - lo:src_hi - lo] = np.asarray(fine[b], np.float32)[src_lo:src_hi]
    cb = np.asarray(coarse[b], np.float32)
    rows = (r0 // 2 - 1 + np.arange(34)) % 32
    coarse_rot = cb[rows].reshape(34 * 32, C)

    we = np.zeros((128, 8, 2), np.float32)
    wo = np.zeros((128, 8, 2), np.float32)
    for blk in range(8):
        for i in range(4):
            se = blk * 4 + i
            y = (r0 - 1 + 2 * se) % H
            pa, pb = (1.0, 0.0) if y == H - 1 else (0.75, 0.25)
            we[32 * i:32 * (i + 1), blk, 0] = pa
            we[32 * i:32 * (i + 1), blk, 1] = pb
            so = blk * 4 + i
            y = (r0 + 2 * so) % H
            pa, pb = (0.0, 1.0) if y == 0 else (0.25, 0.75)
            wo[32 * i:32 * (i + 1), blk, 0] = pa
            wo[32 * i:32 * (i + 1), blk, 1] = pb

    ym = np.ones((128, 34 * W), np.float32)
    zslot = 0 if r0 == 0 else 33
    ym[:, zslot * W:(zslot + 1) * W] = 0.0

    pos = np.arange(POS, dtype=np.float32)
    yg = r0 + pos // W
    xg = pos % W
    kyr = np.tile(wd["ky"], DG)
    kxr = np.tile(wd["kx"], DG)
    cy = (yg[None, :] + kyr[:, None]).astype(np.float32)
    cx = (xg[None, :] + kxr[:, None]).astype(np.float32)
    cr = np.full((72, 1), float(r0 - 1), np.float32)

    import ml_dtypes
    return {
        "fine_full": fb, "fine_halo": halo.reshape(HALO, C),
        "coarse_rot": coarse_rot,
        "w_att": wd["w_att"], "w_sel": wd["w_sel"],
        "wpb": wd["wpb"],
        "dcn_wb": wd["dcn_wb"], "dcn_b": wd["dcn_b"].reshape(F, 1),
        "ohb": wd["ohb"],
        "cy": cy, "cx": cx, "cr": cr,
        "ymb": ym.astype(ml_dtypes.bfloat16),
        "we": we.reshape(128, 16), "wo": wo.reshape(128, 16),
        "idt": np.eye(128, dtype=np.float32),
        "ones": np.ones((128, 1), np.float32),
    }


# ---------------------------------------------------------------- device
def _build_nc():
    nc = bacc.Bacc("TRN2", target_bir_lowering=False, debug=False)
    dt = nc.dram_tensor
    fine_full = dt("fine_full", [H * W, C], FP32, kind="ExternalInput").ap()
    fine_halo = dt("fine_halo", [HALO, C], FP32, kind="ExternalInput").ap()
    coarse_rot = dt("coarse_rot", [34 * 32, C], FP32, kind="ExternalInput").ap()
    w_att = dt("w_att", [C, C], FP32, kind="ExternalInput").ap()
    w_sel = dt("w_sel", [C, F], FP32, kind="ExternalInput").ap()
    wpb = dt("wpb", [KK, 2 * F, OMF], BF16, kind="ExternalInput").ap()
    dcn_wb = dt("dcn_wb", [KK, C, F], BF16, kind="ExternalInput").ap()
    dcn_b = dt("dcn_b", [F, 1], FP32, kind="ExternalInput").ap()
    ohb = dt("ohb", [2, KK, 72, 128], BF16, kind="ExternalInput").ap()
    cy_d = dt("cy", [72, POS], FP32, kind="ExternalInput").ap()
    cx_d = dt("cx", [72, POS], FP32, kind="ExternalInput").ap()
    cr_d = dt("cr", [72, 1], FP32, kind="ExternalInput").ap()
    ymb_d = dt("ymb", [128, HALO], BF16, kind="ExternalInput").ap()
    we_d = dt("we", [128, 16], FP32, kind="ExternalInput").ap()
    wo_d = dt("wo", [128, 16], FP32, kind="ExternalInput").ap()
    idt_d = dt("idt", [128, 128], FP32, kind="ExternalInput").ap()
    ones_d = dt("ones", [128, 1], FP32, kind="ExternalInput").ap()
    out_d = dt("out", [POS, C], FP32, kind="ExternalOutput").ap()
    idx_scr = dt("idx_scr", [2, 4, 2, 16, KK, 2, 64], I16).ap()

    with TileCtx(nc) as tc, ExitStack() as ctx:
        v, s, pe, gp = nc.vector, nc.scalar, nc.tensor, nc.gpsimd
        pool = lambda name, bufs: ctx.enter_context(tc.tile_pool(name=name, bufs=bufs))

        cst = pool("cst", 1)
        idt = cst.tile([128, 128], FP32); nc.sync.dma_start(idt[:], idt_d)
        idtb = cst.tile([128, 128], BF16)
        v.tensor_copy(idtb[:], idt[:])
        ones = cst.tile([128, 1], FP32); nc.sync.dma_start(ones[:], ones_d)
        crt = cst.tile([72, 1], FP32); nc.sync.dma_start(crt[:], cr_d)
        dcnbt = cst.tile([128, 2], FP32)
        nc.sync.dma_start(dcnbt[:], dcn_b.rearrange("(c p) one -> p (c one)", c=2))
        dwt = cst.tile([128, 2 * KK * F], BF16)
        nc.sync.dma_start(
            dwt[:].rearrange("p (k f) -> p k f", k=2 * KK),
            dcn_wb.rearrange("k (c p) f -> p (k c) f", p=128))
        oht = cst.tile([72, 2 * KK * 128], BF16)
        nc.sync.dma_start(
            oht[:].rearrange("r (t k p) -> r t k p", t=2, k=KK),
            ohb.rearrange("t k r p -> r t k p"))

        # persistent tiles
        big = pool("big", 1)
        fcal = [big.tile([128, HALO], BF16, name=f"fcal{i}", tag=f"fcal{i}")
                for i in range(2)]
        W4p = [big.tile([72, 4 * PPOS], BF16, name=f"w4p{_i}")
               for _i in range(2)]
        IUSp = [big.tile([72, PPOS], I16, name=f"ius{_i}", tag=f"ius{_i}")
                for _i in range(2)]
        IWp = [[big.tile([128, KK * 64], I16, name=f"iw{_i}_{_p}")
                for _p in range(2)] for _i in range(2)]

        if True:
            dyx = ctx.enter_context(tc.tile_pool(name="dyxp", bufs=1))
            DYc = [dyx.tile([72, 512], BF16, name=f"dy{i}") for i in range(4)]
            DXc = [dyx.tile([72, 512], BF16, name=f"dx{i}") for i in range(4)]
            MSc = [dyx.tile([72, 512], BF16, name=f"ms{i}") for i in range(4)]

            with tc.tile_pool(name="etp", bufs=1) as etp_pool:
                ETp = [etp_pool.tile([128, H * W], BF16, name=f"etp{i}",
                                     tag=f"etp{i}") for i in range(2)]

                with tc.tile_pool(name="stage1", bufs=1) as st1, \
                     tc.tile_pool(name="ld", bufs=4) as ld, \
                     tc.tile_pool(name="tp_ps", bufs=2, space="PSUM") as tp_ps:
                    fht = [st1.tile([128, HALO], BF16, name=f"fht{i}",
                                    tag=f"fht{i}") for i in range(2)]
                    wselb = st1.tile([128, 2 * F], BF16)
                    wet = st1.tile([128, 16], FP32); nc.sync.dma_start(wet[:], we_d)
                    wot = st1.tile([128, 16], FP32); nc.sync.dma_start(wot[:], wo_d)
                    gap = st1.tile([128, 2], FP32)
                    sct = st1.tile([128, 2], FP32)

                    # ---- GAP via ones-matmul over fine_full
                    with tc.tile_pool(name="gap_ps", bufs=1, space="PSUM") as gpp:
                        gap_ps = [gpp.tile([128, 1], FP32, name=f"gps{i}")
                                  for i in range(2)]
                        for i in range(32):
                            t = ld.tile([128, C], FP32, tag="fln")
                            nc.sync.dma_start(t[:],
                                              fine_full[128 * i:128 * (i + 1), :])
                            for h_ in range(2):
                                pe.matmul(gap_ps[h_][:],
                                          t[:, 128 * h_:128 * (h_ + 1)],
                                          ones[:], start=(i == 0), stop=(i == 31))
                        for h_ in range(2):
                            s.copy(gap[:, h_:h_ + 1], gap_ps[h_][:])

                    # ---- attn -> wselb
                    with tc.tile_pool(name="att", bufs=2) as attp, \
                         tc.tile_pool(name="att_ps", bufs=1, space="PSUM") as atpp:
                        for fc in range(2):
                            aps = atpp.tile([128, 1], FP32, tag="aps")
                            for cc in range(2):
                                wt = attp.tile([128, 128], FP32, tag="watt")
                                nc.sync.dma_start(
                                    wt[:], w_att[128 * cc:128 * (cc + 1),
                                                 128 * fc:128 * (fc + 1)])
                                pe.matmul(aps[:], wt[:], gap[:, cc:cc + 1],
                                          start=(cc == 0), stop=(cc == 1))
                            s.activation(sct[:, fc:fc + 1], aps[:], AF.Sigmoid)
                        v.tensor_scalar(sct[:], sct[:], 1.0, None, OP.add)
                        for cc in range(2):
                            wt = attp.tile([128, F], FP32, tag="wselld")
                            nc.sync.dma_start(wt[:],
                                              w_sel[128 * cc:128 * (cc + 1), :])
                            v.tensor_scalar(wselb[:, F * cc:F * (cc + 1)],
                                            wt[:], sct[:, cc:cc + 1], None,
                                            OP.mult)

                    # ---- transpose fine_halo -> fht (bf16)
                    for i in range(17):
                        t = ld.tile([128, C], FP32, tag="fhl")
                        nc.sync.dma_start(t[:], fine_halo[128 * i:128 * (i + 1), :])
                        for cc in range(2):
                            ps = tp_ps.tile([128, 128], FP32, tag="tp")
                            pe.transpose(ps[:], t[:, 128 * cc:128 * (cc + 1)],
                                         idt[:])
                            s.copy(fht[cc][:, 128 * i:128 * (i + 1)], ps[:])

                    # ---- coarse: y-pass, transpose, x-pass -> ETp (plain)
                    with tc.tile_pool(name="yeyo", bufs=1) as yeyo:
                        ye = yeyo.tile([128, 8 * C], FP32)
                        yo = yeyo.tile([128, 8 * C], FP32)
                        yev = ye[:].rearrange("p (i c) -> p i c", i=8)
                        yov = yo[:].rearrange("p (i c) -> p i c", i=8)
                        with tc.tile_pool(name="crot", bufs=1) as crotp:
                            crA = crotp.tile([128, 8 * C], FP32)
                            crB = crotp.tile([128, 9 * C], FP32)
                            crAv = crA[:].rearrange("p (i c) -> p i c", i=8)
                            crBv = crB[:].rearrange("p (i c) -> p i c", i=9)
                            nc.sync.dma_start(
                                crAv,
                                coarse_rot[:1024].rearrange("(i p) c -> p i c",
                                                            p=128))
                            nc.sync.dma_start(
                                crBv[:, 0:8, :],
                                coarse_rot[32:1056].rearrange("(i p) c -> p i c",
                                                              p=128))
                            nc.sync.dma_start(crBv[0:32, 8, :],
                                              coarse_rot[1056:1088])
                            for blk in range(8):
                                for (dstv, wtile) in ((yev, wet), (yov, wot)):
                                    tb = crotp.tile([128, C], FP32, tag="yt")
                                    v.tensor_scalar(
                                        tb[:], crBv[:, blk, :],
                                        wtile[:, 2 * blk + 1:2 * blk + 2],
                                        None, OP.mult)
                                    v.scalar_tensor_tensor(
                                        dstv[:, blk, :], crAv[:, blk, :],
                                        wtile[:, 2 * blk:2 * blk + 1], tb[:],
                                        OP.mult, OP.add)
                        with tc.tile_pool(name="cupf", bufs=1) as cupf:
                            cmid = [cupf.tile([128, H * 32], BF16, name=f"cmid{i}",
                                              tag=f"cmid{i}") for i in range(2)]
                            for ysrc, par in ((yev, 0), (yov, 1)):
                                for i in range(8):
                                    for cc in range(2):
                                        ps = tp_ps.tile([128, 128], FP32, tag="tp")
                                        pe.transpose(
                                            ps[:],
                                            ysrc[:, i, 128 * cc:128 * (cc + 1)],
                                            idt[:])
                                        dst = cmid[cc][:].rearrange(
                                            "p (s2 two x) -> p s2 two x",
                                            two=2, x=32)
                                        s.copy(dst[:, 4 * i:4 * (i + 1), par, :],
                                               ps[:])
                            for cc in range(2):
                                mid = cmid[cc][:].rearrange("p (s x) -> p s x",
                                                            s=H)
                                out3 = ETp[cc][:].rearrange("p (s x) -> p s x",
                                                            s=H)
                                a = cupf.tile([128, H * 32], BF16, tag="xq")
                                v.tensor_scalar(a[:], cmid[cc][:], 0.25, None,
                                                OP.mult)
                                a3 = a[:].rearrange("p (s x) -> p s x", s=H)
                                v.scalar_tensor_tensor(out3[:, :, 2:64:2],
                                                       mid[:, :, 1:], 0.75,
                                                       a3[:, :, 0:31],
                                                       OP.mult, OP.add)
                                v.tensor_copy(out3[:, :, 0:1], mid[:, :, 0:1])
                                v.scalar_tensor_tensor(out3[:, :, 1:63:2],
                                                       mid[:, :, 0:31], 0.75,
                                                       a3[:, :, 1:32],
                                                       OP.mult, OP.add)
                                v.tensor_copy(out3[:, :, 63:64], mid[:, :, 31:32])

                    # ---- fine_cal = wselb.T @ fht  (bf16 out)
                    with tc.tile_pool(name="fc_ps", bufs=2, space="PSUM") as fc_ps:
                        for fc_ in range(2):
                            for pblk in range(5):
                                n0 = 512 * pblk
                                n1 = min(n0 + 512, HALO)
                                ps = fc_ps.tile([128, 512], FP32, tag="fc")
                                for cc in range(2):
                                    pe.matmul(ps[:, :n1 - n0],
                                              wselb[:, F * cc + 128 * fc_:
                                                    F * cc + 128 * fc_ + 128],
                                              fht[cc][:, n0:n1],
                                              start=(cc == 0), stop=(cc == 1))
                                s.copy(fcal[fc_][:, n0:n1], ps[:, :n1 - n0])

                # ---- build d=4 quad table from ETp (stage1 freed, ETp alive)
                ET4 = [big.tile([128, H * W * 4], BF16, name=f"et4_{i}",
                                tag=f"et4_{i}") for i in range(2)]
                for cc in range(2):
                    e4v = ET4[cc][:].rearrange("p (q d) -> p q d", d=4)
                    for j, off in enumerate((0, 1, 64, 65)):
                        n = H * W - off
                        v.tensor_copy(e4v[:, 0:n, j], ETp[cc][:, off:H * W])
                        if off:
                            v.tensor_copy(e4v[:, n:H * W, j],
                                          ETp[cc][:, 0:off])

            # ---- om conv + index math share one pool scope so their SBUF
            # slabs coexist and the math overlaps the om conv / main loop
            mth = ctx.enter_context(tc.tile_pool(name="mth", bufs=1))
            cycx = ctx.enter_context(tc.tile_pool(name="cycx", bufs=1))
            with tc.tile_pool(name="ppad", bufs=1) as ppad, \
                 tc.tile_pool(name="om_ps", bufs=2, space="PSUM") as om_ps:
                ymt = ppad.tile([128, HALO], BF16)
                nc.sync.dma_start(ymt[:], ymb_d)
                ptf = [ppad.tile([128, 34 * 66], BF16, name=f"ptf{i}",
                                 tag=f"ptf{i}") for i in range(2)]
                ptc = [ppad.tile([128, 34 * 66], BF16, name=f"ptc{i}",
                                 tag=f"ptc{i}") for i in range(2)]
                for i in range(2):
                    gp.memset(ptf[i][:], 0.0)
                    v.tensor_tensor(
                        ptf[i][:].rearrange("p (y x) -> p y x",
                                            y=34)[:, :, 1:65],
                        fcal[i][:].rearrange("p (y x) -> p y x", y=34),
                        ymt[:].rearrange("p (y x) -> p y x", y=34),
                        OP.mult)
                    gp.memset(ptc[i][:], 0.0)
                    ev = ET4[i][:].rearrange("p (s x d) -> p s x d",
                                             s=H, d=4)
                    v.tensor_tensor(
                        ptc[i][:].rearrange("p (y x) -> p y x",
                                            y=34)[:, :, 1:65],
                        ev[:, 0:34, :, 0],
                        ymt[:].rearrange("p (y x) -> p y x", y=34),
                        OP.mult)
                womt = ppad.tile([128, KK * 4 * OMF], BF16)
                nc.sync.dma_start(
                    womt[:].rearrange("p (k c f) -> p k c f", k=KK, c=4),
                    wpb.rearrange("k (c p) f -> p k c f", p=128))
                womv = womt[:].rearrange("p (k c f) -> p k c f", k=KK, c=4)
                for pblk in range(4):
                    psD = om_ps.tile([72, 512], FP32, tag="omD")
                    psX = om_ps.tile([72, 512], FP32, tag="omX")
                    psM = om_ps.tile([72, 512], FP32, tag="omM")
                    for k in range(KK):
                        dy, dx = k // 3 - 1, k % 3 - 1
                        for cc in range(4):
                            pt_ = (ptf[cc] if cc < 2 else ptc[cc - 2])
                            rv = pt_[:].rearrange("p (y x) -> p y x",
                                                  y=34)[
                                :, 1 + 8 * pblk + dy:9 + 8 * pblk + dy,
                                1 + dx:65 + dx]
                            wv = womv[:, k, cc, :]
                            first = (k == 0 and cc == 0)
                            last = (k == KK - 1 and cc == 3)
                            pe.matmul(psD[:], wv[:, 0:72], rv,
                                      start=first, stop=last)
                            pe.matmul(psX[:], wv[:, 72:144], rv,
                                      start=first, stop=last)
                            pe.matmul(psM[:], wv[:, 144:216], rv,
                                      start=first, stop=last)
                    s.copy(DYc[pblk][:], psD[:])
                    s.copy(DXc[pblk][:], psX[:])
                    s.activation(MSc[pblk][:], psM[:], AF.Sigmoid)

            # ---- index & weight math
            if True:
                CH = 512
                tn = lambda nm: mth.tile([72, CH], FP32, name=nm, tag=nm)
                tf, tg, XP, FL, FR, X1 = (tn(x) for x in
                                          ("tf", "tg", "XP", "FL", "FR", "X1"))
                R0c, VA, VB, WY0, WY1, SSc, A0, A1, WR0, WR1 = (
                    tn(x) for x in ("R0c", "VA", "VB", "WY0", "WY1",
                                    "SSc", "A0", "A1", "WR0", "WR1"))
                SM = A0

                def floor_(dst, srcp):
                    v.tensor_copy(tg[:].bitcast(I32), srcp[:])
                    v.tensor_copy(tf[:], tg[:].bitcast(I32))
                    v.tensor_tensor(tg[:], tf[:], srcp[:], OP.is_gt)
                    v.tensor_tensor(dst[:], tf[:], tg[:], OP.subtract)

                for ch in range(POS // CH):
                    c0 = CH * ch
                    phc, lo = ch // 2, 512 * (ch % 2)
                    W4v = W4p[phc][:].rearrange("p (n t) -> p n t", t=4)
                    sl = slice(lo, lo + CH)
                    cyt = cycx.tile([72, CH], FP32, tag="cyt")
                    nc.sync.dma_start(cyt[:], cy_d[:, c0:c0 + CH])
                    cxt = cycx.tile([72, CH], FP32, tag="cxt")
                    nc.sync.dma_start(cxt[:], cx_d[:, c0:c0 + CH])
                    # y side
                    v.tensor_tensor(XP[:], DYc[ch][:], cyt[:], OP.add)
                    floor_(FL, XP)
                    v.tensor_tensor(FR[:], XP[:], FL[:], OP.subtract)
                    v.tensor_scalar(X1[:], FL[:], 1.0, None, OP.add)
                    v.tensor_scalar(VA[:], FL[:], 0.0, None, OP.is_ge)
                    v.tensor_scalar(tg[:], FL[:], 63.0, None, OP.is_le)
                    v.tensor_tensor(VA[:], VA[:], tg[:], OP.mult)
                    v.tensor_scalar(VB[:], X1[:], 0.0, None, OP.is_ge)
                    v.tensor_scalar(tg[:], X1[:], 63.0, None, OP.is_le)
                    v.tensor_tensor(VB[:], VB[:], tg[:], OP.mult)
                    v.tensor_scalar(tf[:], FR[:], -1.0, 1.0, OP.mult, OP.add)
                    v.tensor_tensor(tf[:], tf[:], VA[:], OP.mult)
                    v.tensor_tensor(WY0[:], tf[:], MSc[ch][:], OP.mult)
                    v.tensor_tensor(tf[:], FR[:], VB[:], OP.mult)
                    v.tensor_tensor(WY1[:], tf[:], MSc[ch][:], OP.mult)
                    # row-collapse: SM = (FL in [0,62]); WR1 = SM*WY1;
                    # WR0 = WY0 + WY1 - WR1
                    v.tensor_scalar(SM[:], FL[:], 0.0, None, OP.is_ge)
                    v.tensor_scalar(tg[:], FL[:], 62.0, None, OP.is_le)
                    v.tensor_tensor(SM[:], SM[:], tg[:], OP.mult)
                    v.tensor_tensor(WR1[:], SM[:], WY1[:], OP.mult)
                    v.tensor_tensor(WR0[:], WY0[:], WY1[:], OP.add)
                    v.tensor_tensor(WR0[:], WR0[:], WR1[:], OP.subtract)
                    v.tensor_scalar(R0c[:], FL[:], 0.0, 63.0, OP.max, OP.min)
                    # x side
                    v.tensor_tensor(XP[:], DXc[ch][:], cxt[:], OP.add)
                    floor_(FL, XP)
                    v.tensor_tensor(FR[:], XP[:], FL[:], OP.subtract)
                    v.tensor_scalar(X1[:], FL[:], 1.0, None, OP.add)
                    v.tensor_scalar(VA[:], FL[:], 0.0, None, OP.is_ge)
                    v.tensor_scalar(tg[:], FL[:], 63.0, None, OP.is_le)
                    v.tensor_tensor(VA[:], VA[:], tg[:], OP.mult)
                    v.tensor_scalar(VB[:], X1[:], 0.0, None, OP.is_ge)
                    v.tensor_scalar(tg[:], X1[:], 63.0, None, OP.is_le)
                    v.tensor_tensor(VB[:], VB[:], tg[:], OP.mult)
                    v.tensor_scalar(tf[:], FR[:], -1.0, 1.0, OP.mult, OP.add)
                    v.tensor_tensor(A0[:], tf[:], VA[:], OP.mult)
                    v.tensor_tensor(A1[:], FR[:], VB[:], OP.mult)
                    v.tensor_scalar(SSc[:], FL[:], 0.0, 62.0, OP.max, OP.min)
                    for sidx in range(2):
                        if sidx == 0:
                            v.tensor_tensor(tf[:], SSc[:], FL[:], OP.is_equal)
                            v.tensor_tensor(tg[:], SSc[:], X1[:], OP.is_equal)
                        else:
                            v.tensor_scalar(VA[:], SSc[:], 1.0, None, OP.add)
                            v.tensor_tensor(tf[:], VA[:], FL[:], OP.is_equal)
                            v.tensor_tensor(tg[:], VA[:], X1[:], OP.is_equal)
                        v.tensor_tensor(tf[:], tf[:], A0[:], OP.mult)
                        v.tensor_tensor(tg[:], tg[:], A1[:], OP.mult)
                        v.tensor_tensor(tf[:], tf[:], tg[:], OP.add)
                        v.tensor_tensor(W4v[:, sl, sidx], WR0[:], tf[:],
                                        OP.mult)
                        v.tensor_tensor(W4v[:, sl, 2 + sidx], WR1[:], tf[:],
                                        OP.mult)
                    # rotated slot index -> IUS (wrapped free order)
                    v.tensor_scalar(tf[:], R0c[:], crt[:, 0:1], None,
                                    OP.subtract)
                    v.tensor_scalar(tg[:], tf[:], 0.0, None, OP.is_lt)
                    v.scalar_tensor_tensor(tf[:], tg[:], 64.0, tf[:],
                                           OP.mult, OP.add)
                    v.tensor_scalar(tg[:], tf[:], 64.0, None, OP.is_ge)
                    v.scalar_tensor_tensor(tf[:], tg[:], -64.0, tf[:],
                                           OP.mult, OP.add)
                    v.tensor_scalar(tf[:], tf[:], 64.0, None, OP.mult)
                    v.tensor_tensor(tf[:], tf[:], SSc[:], OP.add)
                    ncol = CH // 16
                    dstv = IUSp[phc][:].rearrange(
                        "p (s c) -> p s c", s=16)[
                        :, :, ncol * (ch % 2):ncol * (ch % 2 + 1)].rearrange(
                        "p s c -> p c s")
                    v.tensor_copy(dstv,
                                  tf[:].rearrange("p (c s) -> p c s",
                                                  s=16))

        alp = pool("alp", 1)
        AL = [alp.tile([128, POS], BF16, name=f"al{i}", tag=f"al{i}")
              for i in range(2)]

        # ---- idx bounce: duplicated wrapped layout, split by phase so the
        # ph0 main loop starts after only math chunks 0-1
        for ph_ in range(2):
            for t_ in range(2):
                for d_ in range(2):
                    for g_ in range(4):
                        nc.sync.dma_start(
                            idx_scr[t_, g_, d_, :, :, ph_, :].rearrange(
                                "p k col -> k p col"),
                            IUSp[ph_][36 * t_ + 9 * g_:
                                      36 * t_ + 9 * (g_ + 1), :]
                            .rearrange("k (p col) -> k p col", p=16))
            for t_ in range(2):
                nc.sync.dma_start(
                    IWp[t_][ph_][:].rearrange("p (k col) -> p k col", k=KK),
                    idx_scr[t_].rearrange(
                        "g d p k ph col -> (g d p) k ph col")[:, :, ph_, :])

        # ---- main loop
        fin_ps = ctx.enter_context(
            tc.tile_pool(name="fin_ps", bufs=2, space="PSUM"))
        fin_sb = ctx.enter_context(tc.tile_pool(name="fin_sb", bufs=2))
        with tc.tile_pool(name="gat", bufs=2) as gat, \
             tc.tile_pool(name="wrep", bufs=1) as wrep, \
             tc.tile_pool(name="rep_ps", bufs=2, space="PSUM") as rep_ps, \
             tc.tile_pool(name="dcn_ps", bufs=1, space="PSUM") as dcn_ps, \
             tc.tile_pool(name="val", bufs=1) as val_p:
            dwv = dwt[:].rearrange("p (k f) -> p k f", k=2 * KK)

            ohv = oht[:].rearrange("r (t k p) -> r t k p", t=2, k=KK)
            for ph in range(NPH):
                p0 = PPOS * ph
                dps = [dcn_ps.tile([128, 512], FP32, name=f"dcn{ph}_{i}",
                                   tag=f"dcn{i}") for i in range(4)]
                for t_ in range(2):
                    for k in range(KK):
                        wre = wrep.tile([128, 4 * PPOS], BF16, tag="wr")
                        for ns in range(8):
                            ps = rep_ps.tile([128, 512], FP32, tag="rep")
                            pe.matmul(ps[:], ohv[:, t_, k, :],
                                      W4p[ph][:, 512 * ns:512 * (ns + 1)],
                                      start=True, stop=True)
                            s.copy(wre[:, 512 * ns:512 * (ns + 1)], ps[:])
                        g4 = gat.tile([128, PPOS, 4], BF16, tag="g4")
                        gp.ap_gather(
                            g4[:],
                            ET4[t_][:].rearrange("p (q d) -> p q d", d=4),
                            IWp[t_][ph][:].rearrange(
                                "p (k col) -> p k col", k=KK)[:, k, :],
                            channels=128, num_elems=H * W, d=4,
                            num_idxs=PPOS)
                        P0 = val_p.tile([128, 4 * PPOS], BF16, tag="p0")
                        val = val_p.tile([128, PPOS], BF16, tag="val")
                        g4f = g4[:].rearrange("p n d -> p (n d)")
                        v.tensor_tensor(P0[:], g4f, wre[:], OP.mult)
                        p44 = P0[:].rearrange("p (n f) -> p n f", f=4)
                        v.tensor_tensor(val[:], p44[:, :, 0], p44[:, :, 1],
                                        OP.add)
                        v.tensor_tensor(val[:], val[:], p44[:, :, 2], OP.add)
                        v.tensor_tensor(val[:], val[:], p44[:, :, 3], OP.add)
                        first = (t_ == 0 and k == 0)
                        last = (t_ == 1 and k == KK - 1)
                        for fc_ in range(2):
                            for ns in range(2):
                                pe.matmul(
                                    dps[2 * fc_ + ns][:],
                                    dwv[:, 2 * k + t_,
                                        128 * fc_:128 * (fc_ + 1)],
                                    val[:, 512 * ns:512 * (ns + 1)],
                                    start=first, stop=last)
                for fc_ in range(2):
                    for ns in range(2):
                        s.activation(
                            AL[fc_][:, p0 + 512 * ns:p0 + 512 * (ns + 1)],
                            dps[2 * fc_ + ns][:], AF.Relu,
                            bias=dcnbt[:, fc_:fc_ + 1])
                # early epilogue for this phase: fcal add + transpose + out
                for fc_ in range(2):
                    v.tensor_tensor(AL[fc_][:, p0:p0 + PPOS],
                                    AL[fc_][:, p0:p0 + PPOS],
                                    fcal[fc_][:, W + p0:W + p0 + PPOS],
                                    OP.add)
                for i in range(8):
                    it = 8 * ph + i
                    ot = fin_sb.tile([128, C], FP32, tag="ot")
                    for fc_ in range(2):
                        ps = fin_ps.tile([128, 128], BF16, tag="fin")
                        pe.transpose(
                            ps[:], AL[fc_][:, p0 + 128 * i:p0 + 128 * (i + 1)],
                            idtb[:])
                        s.copy(ot[:, 128 * fc_:128 * (fc_ + 1)], ps[:])
                    nc.sync.dma_start(out_d[128 * it:128 * (it + 1), :], ot[:])

    nc.compile()
    return nc


TileCtx = tile.TileContext
_NC_CACHE = None


def _get_nc():
    global _NC_CACHE
    if _NC_CACHE is None:
        _NC_CACHE = _build_nc()
    return _NC_CACHE


def kernel(fine, coarse, attend_w, select_w, offset_w, om_w, om_b, dcn_w,
           dcn_b, _trace=False, _trace_kwargs=None):
    wd = _prep_weights(np.asarray(attend_w), np.asarray(select_w),
                       np.asarray(offset_w), np.asarray(om_w),
                       np.asarray(om_b), np.asarray(dcn_w), np.asarray(dcn_b))
    in_maps = [_core_inputs(c, np.asarray(fine), np.asarray(coarse), wd)
               for c in range(NCORES)]
    nc = _get_nc()
    kw = {}
    if _trace:
        import tempfile
        import concourse.bass_utils as _bu
        _bu.upload_artifacts = lambda d: d
        tdir = tempfile.mkdtemp(prefix="bass_trace_")
        kernel._last_trace_dir = tdir
        kw = dict(trace=True, trace_kwargs=_trace_kwargs or {}, tmpdir=tdir)
    res = run_bass_kernel_spmd(nc, in_maps, list(range(NCORES)), **kw)
    out = np.zeros((B, H, W, C), np.float32)
    for c in range(NCORES):
        b, half = c // 2, c % 2
        out[b, 32 * half:32 * half + 32] = res.results[c]["out"].reshape(32, W, C)
    if _trace:
        kernel._last_exec_ns = res.exec_time_ns
        kernel._last_trace = res.instructions_and_trace
    return out
